# revision 19
# baseline (speedup 1.0000x reference)
"""Trainium2 Bass kernel for nn_AutoCorr2D.

Computation (per sample):
  f   = conv3x3(x, w_ext, pad=1) + b_ext            # [CC=128, 64, 64]
  corr[c,i,j,k] = f[c,i,j] * fpad[c, i+u-2, j+v-2]  # 5x5 window products
  out[o,i,j]    = sum_{c,k} w_reg[o,c,k] * corr[c,i,j,k] + b_reg[o]

Sharding: data-parallel over batch B=8 across 8 NeuronCores (one sample per
core); conv weights replicated.

All compute runs in bf16 (inputs host-cast; PSUM accumulation stays f32);
measured end-to-end rel err ~5e-3 vs the f32 reference (tolerance 2e-2).
bf16 buys: 2x DVE rate for the product maps (2x_1p mode needs all-2-byte
packed operands), halved DMA traffic, and PE weight loads short enough to
hide entirely under the previous matmul's stream (f32r paid ~27ns/matmul).

Per-core implementation (steady-state ~216.5ns per 512-column matmul, vs
the 213.3ns bf16 streaming floor at 2.4GHz):
  stage 1: implicit GEMM - 18 accumulating matmuls (cin tile x 3x3 tap)
           per 512-pixel chunk, reading shifted views of the host-padded
           x image; bias folded into the PSUM->SBUF copy (ScalarE
           Identity) writing bf16 fpad.
  stage 2: product symmetry: P_{a,b}[y,x] = f[y,x]*f[y+a,x+b] serves both
           tap (a,b) (read at [i,j]) and tap (-a,-b) (read at [i-a,j-b]),
           so only 13 of 25 product maps are computed per 2-chunk group
           (ScalarE Square for (0,0), VectorE bf16 for the rest).  Then 25
           accumulating matmuls per chunk (K=128 channels per tap) into
           PSUM[64,512]; bias-copy + out DMA overlap the next chunk's
           GEMM, and the final chunk's copy/DMA is split in halves to
           shorten the post-matmul serial tail.

Startup/DMA choreography (the other measured bottlenecks):
  - DMA completion semaphores post ~1.5us after the data on an idle HWDGE
    queue and 3-5us on a busy one; the first x band + first wext block
    gate the first real matmul (~10.7-11.9us), so they lead the two HWDGE
    queues (sync: x bands for cin tile 0; scalar: wext halves + bands for
    cin tile 1, which aren't read until matmul #19 of a chunk).
  - x bands are DISJOINT slabs (overlap creates WAW deps that serialize
    the queue on completion semaphores) and pair-aligned (splits at xpad
    rows 18/34/50) so chunk pair p is gated by band p only.  x arrives
    host-padded ([128, 66, 66] bf16, zero borders baked in) making every
    band DMA fully contiguous - 128 descriptors instead of ~2200 strided
    128B ones, which previously made the DMA descriptor-rate the
    bottleneck.
  - 9 warm-up matmuls on a GpSimd-zeroed scratch bridge the DVFS/HAM
    p-state ramp from the end of the framework preamble (~7.2us) to the
    first real matmul; a PE idle gap >0.5us here resets the ramp and
    costs ~1us of half-rate matmuls.
  (Remaining fixed costs: ~7us end-of-NEFF semaphore-file clear parade +
  ~1.2us barrier + ~1.8us final out-DMA semaphore drain - all emitted by
  the NEFF compiler / DGE hardware, not reachable from kernel code.)
"""

import numpy as np
import ml_dtypes

from concourse import bacc, mybir, tile
from concourse.bass_utils import run_bass_kernel_spmd

B, CIN, H, W = 8, 256, 64, 64
CC, COUT = 128, 64
HW = H * W
NCORES = 8

NCHUNK = 8           # pixel chunks per image
CROWS = H // NCHUNK  # rows per chunk (8) -> N = 512 pixels
NPX = CROWS * W      # 512
NGRP = 4             # product-map groups (2 chunks each)
GROWS = 2 * CROWS    # 16

XP = W + 2           # xpad cols (pad=1)
XR = H + 2           # xpad rows
FP = W + 4           # fpad cols (pad=2)
FR = H + 4           # fpad rows
FTAIL = 72           # guard tail so shifted product reads stay in-bounds

# The 13 "upper half" taps; (a,b) also serves tap (-a,-b) via a shifted read.
SYM = [(0, 0), (0, 1), (0, 2),
       (1, -2), (1, -1), (1, 0), (1, 1), (1, 2),
       (2, -2), (2, -1), (2, 0), (2, 1), (2, 2)]

F32 = mybir.dt.float32
BF16 = mybir.dt.bfloat16
U16 = mybir.dt.uint16
AF = mybir.ActivationFunctionType

NWARM = 9


def build_body(nc, tc, x, wext, wreg, bext, breg, out,
               wext_d, wreg_d, bext_d, breg_d):
    with (
        tc.tile_pool(name="const", bufs=1) as constp,
        tc.tile_pool(name="xpadp", bufs=1) as xpadp,
        tc.tile_pool(name="fpadp", bufs=1) as fpadp,
        tc.tile_pool(name="prodp", bufs=2) as prodp,
        tc.tile_pool(name="outp", bufs=2) as outp,
        tc.tile_pool(name="ps1", bufs=3, space="PSUM") as ps1,
        tc.tile_pool(name="ps2", bufs=4, space="PSUM") as ps2,
        tc.tile_pool(name="warmp", bufs=1, space="PSUM") as warmp,
    ):
        # PE warm-up: dummy matmuls on a GpSimd-zeroed bf16 scratch start as
        # soon as the framework preamble ends, releasing the HAM clock gate
        # before real data arrives.
        wsc = constp.tile([128, NPX], BF16, name="wsc")
        nc.gpsimd.memset(wsc.bitcast(U16), 0)
        wpsum = warmp.tile([128, NPX], F32, name="wpsum")
        for i in range(NWARM):
            nc.tensor.matmul(wpsum, wsc[:, :128], wsc,
                             start=(i == 0), stop=(i == NWARM - 1))

        # x arrives host-padded ([128, 66, 66] with zero borders baked in),
        # so band DMAs are fully contiguous on both sides (128 descriptors
        # instead of ~2200 strided 128B ones) and no border memsets exist.
        xpads = []
        for t in range(2):
            xp = xpadp.tile([128, XR * XP], BF16, name=f"xpad{t}",
                            tag=f"xpad{t}")
            xpads.append(xp.rearrange("p (r c) -> p r c", c=XP))

        # Input DMAs.  Completion-semaphore posting lags the data by ~1.5us
        # on a lightly-loaded HWDGE queue and 3-5us on a congested one, and
        # the first band + first wext block gate stage-1's first matmuls -
        # so the early transfers lead both queues (sync and scalar are the
        # only HWDGE queues).  x bands are DISJOINT (overlap would create
        # WAW deps that serialize the queue on completion semaphores) and
        # pair-aligned: stage-1 pair p reads xpad rows [16p, 16p+18), so
        # with splits at 18/34/50 pair p is gated by band p only.
        def band_dma(q, t, ra, rb):
            dst = xpads[t][:, ra:rb, :]
            src = x[t * 128:(t + 1) * 128, ra * XP:rb * XP]
            nc_q = getattr(nc, q)
            nc_q.dma_start(out=dst, in_=src.rearrange("p (r c) -> p r c", c=XP))

        BSPLIT = (0, 18, 34, 50, 66)

        def wext_dma(lo, hi, q="scalar"):
            getattr(nc, q).dma_start(out=wext[:, lo * 128:hi * 128],
                                     in_=wext_d[:, lo * 128:hi * 128])

        # Both first-matmul gates (band0 t0 + wext blocks 0-8) ride the
        # SAME queue back-to-back: their semaphores post ~0.65us apart, so
        # the start is one queue's bring-up time, not the max of two
        # independent ones.  t=1 transfers (read from matmul #19 of a
        # chunk) lead the scalar queue.
        band_dma("sync", 0, BSPLIT[0], BSPLIT[1])
        wext_dma(0, 9, q="sync")
        band_dma("scalar", 1, BSPLIT[0], BSPLIT[1])
        wext_dma(9, 18)
        band_dma("sync", 0, BSPLIT[1], BSPLIT[2])
        band_dma("scalar", 1, BSPLIT[1], BSPLIT[2])
        nc.scalar.dma_start(out=bext, in_=bext_d)
        nc.scalar.dma_start(out=breg, in_=breg_d)
        band_dma("sync", 0, BSPLIT[2], BSPLIT[3])
        band_dma("scalar", 1, BSPLIT[2], BSPLIT[3])
        band_dma("sync", 0, BSPLIT[3], BSPLIT[4])
        band_dma("scalar", 1, BSPLIT[3], BSPLIT[4])
        nc.scalar.dma_start(out=wreg, in_=wreg_d)

        # ---- padded features (pad=2) + guard tail; borders on GpSimd ----
        fpad = fpadp.tile([128, FR * FP + FTAIL], BF16, name="fpad")
        fr = fpad[:, :FR * FP].rearrange("p (r c) -> p r c", c=FP)
        fi = fpad.bitcast(U16)
        fri = fi[:, :FR * FP].rearrange("p (r c) -> p r c", c=FP)
        nc.gpsimd.memset(fi[:, 0:2 * FP], 0)
        nc.gpsimd.memset(fi[:, (FR - 2) * FP:FR * FP + FTAIL], 0)
        nc.gpsimd.memset(fri[:, 2:FR - 2, 0:2], 0)
        nc.gpsimd.memset(fri[:, 2:FR - 2, FP - 2:FP], 0)

        # ---- stage 1: f = conv3x3(x) + b_ext ----
        # 18 accumulating matmuls (cin tile x 3x3 tap) per 512-pixel chunk;
        # each chunk's bias-copy overlaps the next chunk's matmuls.
        for c in range(NCHUNK):
            ps = ps1.tile([128, NPX], F32, name=f"ps1_{c}", tag="psum1")
            k = 0
            for t in range(2):
                for du in range(3):
                    for dv in range(3):
                        blk = t * 9 + du * 3 + dv
                        lhsT = wext[:, blk * 128:(blk + 1) * 128]
                        rhs = xpads[t][:,
                                       c * CROWS + du:
                                       c * CROWS + du + CROWS,
                                       dv:dv + W]
                        nc.tensor.matmul(ps, lhsT, rhs,
                                         start=(k == 0), stop=(k == 17))
                        k += 1
            dst = fr[:, c * CROWS + 2:c * CROWS + 2 + CROWS, 2:2 + W]
            nc.scalar.activation(dst,
                                 ps.rearrange("p (r c) -> p r c", c=W),
                                 AF.Identity, bias=bext, scale=1.0)

        # ---- stage 2: products (2-chunk groups) + regressor GEMM ----
        for g in range(NGRP):
            # product map for tap (a,b): rows [g*16+2-a, g*16+18) of the
            # (-2-origin) padded product grid, full FP-wide rows
            ptiles = []
            for kk, (a, b) in enumerate(SYM):
                nrows = GROWS + a if kk > 0 else GROWS
                base = (g * GROWS + 2 - (a if kk > 0 else 0)) * FP
                pt = prodp.tile([128, nrows * FP], BF16,
                                name=f"prod{kk}", tag=f"prod{kk}", bufs=2)
                in0 = fpad[:, base:base + nrows * FP]
                in1 = fpad[:, base + a * FP + b:
                           base + a * FP + b + nrows * FP]
                if kk == 0:
                    nc.scalar.activation(pt, in0, AF.Square)
                else:
                    nc.vector.tensor_mul(pt, in0, in1)
                ptiles.append(pt)

            # Regressor GEMM, chunk-outer: each chunk's bias-copy and out
            # DMA overlap the next chunk's GEMM.  bf16 weight loads hide
            # fully under the previous matmul, so per-tap weight reuse
            # buys nothing here.
            for i in (2 * g, 2 * g + 1):
                p8 = (i % 2) * CROWS
                psum2 = ps2.tile([COUT, NPX], F32, name=f"psum2_{i}",
                                 tag="psum2")
                mm = 0
                for kk, (a, b) in enumerate(SYM):
                    pr = ptiles[kk].rearrange("p (r c) -> p r c", c=FP)
                    taps = ([(a, b)] if (a, b) == (0, 0)
                            else [(a, b), (-a, -b)])
                    for (p, q) in taps:
                        if kk == 0:
                            rhs = pr[:, p8:p8 + CROWS, 2:2 + W]
                        elif (p, q) == (a, b):
                            rhs = pr[:, p8 + a:p8 + a + CROWS, 2:2 + W]
                        else:
                            rhs = pr[:, p8:p8 + CROWS, 2 - b:2 - b + W]
                        tidx = (p + 2) * 5 + (q + 2)
                        lhsT = wreg[:, tidx * 64:(tidx + 1) * 64]
                        nc.tensor.matmul(psum2, lhsT, rhs,
                                         start=(mm == 0), stop=(mm == 24))
                        mm += 1

                outt = outp.tile([COUT, NPX], F32, name="outsb",
                                 tag="outsb")
                # Final chunk: halve the copy+DMA so the first half's DMA
                # overlaps the second half's copy (shortens the serial tail
                # after the last matmul).
                halves = ((0, NPX // 2), (NPX // 2, NPX)) if i == 7                     else ((0, NPX),)
                for lo, hi in halves:
                    nc.scalar.activation(outt[:, lo:hi], psum2[:, lo:hi],
                                         AF.Identity, bias=breg, scale=1.0)
                    nc.sync.dma_start(
                        out=out[:, i * NPX + lo:i * NPX + hi],
                        in_=outt[:, lo:hi])


def build_nc():
    nc = bacc.Bacc("TRN2", target_bir_lowering=False, debug=False,
                   num_devices=NCORES)
    x = nc.dram_tensor("x", [CIN, XR * XP], BF16, kind="ExternalInput").ap()
    wext_d = nc.dram_tensor("wext", [128, 18 * 128], BF16,
                            kind="ExternalInput").ap()
    wreg_d = nc.dram_tensor("wreg", [128, 25 * 64], BF16,
                            kind="ExternalInput").ap()
    bext_d = nc.dram_tensor("bext", [128, 1], F32, kind="ExternalInput").ap()
    breg_d = nc.dram_tensor("breg", [64, 1], F32, kind="ExternalInput").ap()
    out = nc.dram_tensor("out", [COUT, HW], F32, kind="ExternalOutput").ap()
    with tile.TileContext(nc) as tc:
        with tc.tile_pool(name="weights", bufs=1) as wp:
            wext = wp.tile([128, 18 * 128], BF16, name="wext_sb")
            wreg = wp.tile([128, 25 * 64], BF16, name="wreg_sb")
            bext = wp.tile([128, 1], F32, name="bext_sb")
            breg = wp.tile([64, 1], F32, name="breg_sb")
            build_body(nc, tc, x, wext, wreg, bext, breg, out,
                       wext_d, wreg_d, bext_d, breg_d)
    nc.compile()
    return nc


def prep_in_maps(x, w_ext, b_ext, w_reg, b_reg):
    x = np.ascontiguousarray(np.asarray(x, dtype=np.float32))
    w_ext = np.asarray(w_ext, dtype=np.float32)
    w_reg = np.asarray(w_reg, dtype=np.float32)
    b_ext = np.asarray(b_ext, dtype=np.float32)
    b_reg = np.asarray(b_reg, dtype=np.float32)

    # lhsT layouts: wext [cin(128-part), (cintile,tap)*cc], wreg [cc, tap*cout]
    w1 = np.transpose(w_ext, (1, 2, 3, 0))          # [CIN, 3, 3, CC]
    wext_p = np.zeros((128, 18, 128), np.float32)
    for t in range(2):
        for du in range(3):
            for dv in range(3):
                wext_p[:, t * 9 + du * 3 + dv, :] = \
                    w1[t * 128:(t + 1) * 128, du, dv, :]
    wext_p = np.ascontiguousarray(wext_p.reshape(128, 18 * 128))
    w2 = np.transpose(w_reg, (1, 2, 3, 0))          # [CC, 5, 5, COUT]
    wreg_p = np.ascontiguousarray(w2.reshape(128, 25 * 64))
    bext_p = np.ascontiguousarray(b_ext.reshape(128, 1))
    breg_p = np.ascontiguousarray(b_reg.reshape(64, 1))

    bf = ml_dtypes.bfloat16
    wext_b = wext_p.astype(bf)
    wreg_b = wreg_p.astype(bf)

    # host-padded x: [CIN, 66, 66] with zero borders (pad=1) baked in
    xpad_h = np.zeros((B, CIN, XR, XP), bf)
    xpad_h[:, :, 1:1 + H, 1:1 + W] = x.reshape(B, CIN, H, W).astype(bf)
    xpad_h = np.ascontiguousarray(xpad_h.reshape(B, CIN, XR * XP))

    return [{
        "x": xpad_h[b],
        "wext": wext_b,
        "wreg": wreg_b,
        "bext": bext_p,
        "breg": breg_p,
    } for b in range(B)]


_NC_CACHE = None


def kernel(x, w_ext, b_ext, w_reg, b_reg):
    global _NC_CACHE
    if _NC_CACHE is None:
        _NC_CACHE = build_nc()
    nc = _NC_CACHE
    in_maps = prep_in_maps(x, w_ext, b_ext, w_reg, b_reg)
    res = run_bass_kernel_spmd(nc, in_maps, list(range(NCORES)))
    return np.stack([res.results[b]["out"].reshape(COUT, H, W)
                     for b in range(B)], axis=0)


# revision 20
# speedup vs baseline: 1.0133x; 1.0133x over previous
"""Trainium2 Bass kernel for nn_AutoCorr2D.

Computation (per sample):
  f   = conv3x3(x, w_ext, pad=1) + b_ext            # [CC=128, 64, 64]
  corr[c,i,j,k] = f[c,i,j] * fpad[c, i+u-2, j+v-2]  # 5x5 window products
  out[o,i,j]    = sum_{c,k} w_reg[o,c,k] * corr[c,i,j,k] + b_reg[o]

Sharding: data-parallel over batch B=8 across 8 NeuronCores (one sample per
core); conv weights replicated.

All compute runs in bf16 (inputs host-cast; PSUM accumulation stays f32);
measured end-to-end rel err ~5e-3 vs the f32 reference (tolerance 2e-2).
bf16 buys: 2x DVE rate for the product maps (2x_1p mode needs all-2-byte
packed operands), halved DMA traffic, and PE weight loads short enough to
hide entirely under the previous matmul's stream (f32r paid ~27ns/matmul).

Per-core implementation (steady-state ~216.5ns per 512-column matmul, vs
the 213.3ns bf16 streaming floor at 2.4GHz):
  stage 1: implicit GEMM - 18 accumulating matmuls (cin tile x 3x3 tap)
           per 512-pixel chunk, reading shifted views of the host-padded
           x image; bias folded into the PSUM->SBUF copy (ScalarE
           Identity) writing bf16 fpad.
  stage 2: product symmetry: P_{a,b}[y,x] = f[y,x]*f[y+a,x+b] serves both
           tap (a,b) (read at [i,j]) and tap (-a,-b) (read at [i-a,j-b]),
           so only 13 of 25 product maps are computed per 2-chunk group
           (ScalarE Square for (0,0), VectorE bf16 for the rest).  Then 25
           accumulating matmuls per chunk (K=128 channels per tap) into
           PSUM[64,512]; bias-copy + out DMA overlap the next chunk's
           GEMM, and the final chunk's copy/DMA is split in halves to
           shorten the post-matmul serial tail.

Startup/DMA choreography (the other measured bottlenecks):
  - DMA completion semaphores post ~1.5us after the data on an idle HWDGE
    queue and 3-5us on a busy one; the first x band + first wext block
    gate the first real matmul (~10.7-11.9us), so they lead the two HWDGE
    queues (sync: x bands for cin tile 0; scalar: wext halves + bands for
    cin tile 1, which aren't read until matmul #19 of a chunk).
  - x bands are DISJOINT slabs (overlap creates WAW deps that serialize
    the queue on completion semaphores) and pair-aligned (splits at xpad
    rows 18/34/50) so chunk pair p is gated by band p only.  x arrives
    host-padded ([128, 66, 66] bf16, zero borders baked in) making every
    band DMA fully contiguous - 128 descriptors instead of ~2200 strided
    128B ones, which previously made the DMA descriptor-rate the
    bottleneck.
  - 9 warm-up matmuls on a GpSimd-zeroed scratch bridge the DVFS/HAM
    p-state ramp from the end of the framework preamble (~7.2us) to the
    first real matmul; a PE idle gap >0.5us here resets the ramp and
    costs ~1us of half-rate matmuls.
  (Remaining fixed costs: ~7us end-of-NEFF semaphore-file clear parade +
  ~1.2us barrier + ~1.8us final out-DMA semaphore drain - all emitted by
  the NEFF compiler / DGE hardware, not reachable from kernel code.)
"""

import numpy as np
import ml_dtypes

from concourse import bacc, mybir, tile
from concourse.bass_utils import run_bass_kernel_spmd

B, CIN, H, W = 8, 256, 64, 64
CC, COUT = 128, 64
HW = H * W
NCORES = 8

NCHUNK = 8           # pixel chunks per image
CROWS = H // NCHUNK  # rows per chunk (8) -> N = 512 pixels
NPX = CROWS * W      # 512
NGRP = 4             # product-map groups (2 chunks each)
GROWS = 2 * CROWS    # 16

XP = W + 2           # xpad cols (pad=1)
XR = H + 2           # xpad rows
FP = W + 4           # fpad cols (pad=2)
FR = H + 4           # fpad rows
FTAIL = 72           # guard tail so shifted product reads stay in-bounds

# The 13 "upper half" taps; (a,b) also serves tap (-a,-b) via a shifted read.
SYM = [(0, 0), (0, 1), (0, 2),
       (1, -2), (1, -1), (1, 0), (1, 1), (1, 2),
       (2, -2), (2, -1), (2, 0), (2, 1), (2, 2)]

F32 = mybir.dt.float32
BF16 = mybir.dt.bfloat16
U16 = mybir.dt.uint16
AF = mybir.ActivationFunctionType

NWARM = 9


def build_body(nc, tc, x, wext, wreg, bext, breg, out,
               wext_d, wreg_d, bext_d, breg_d):
    with (
        tc.tile_pool(name="const", bufs=1) as constp,
        tc.tile_pool(name="xpadp", bufs=1) as xpadp,
        tc.tile_pool(name="fpadp", bufs=1) as fpadp,
        tc.tile_pool(name="prodp", bufs=2) as prodp,
        tc.tile_pool(name="outp", bufs=2) as outp,
        tc.tile_pool(name="ps1", bufs=3, space="PSUM") as ps1,
        tc.tile_pool(name="ps2", bufs=4, space="PSUM") as ps2,
        tc.tile_pool(name="warmp", bufs=1, space="PSUM") as warmp,
    ):
        # PE warm-up: dummy matmuls on a GpSimd-zeroed bf16 scratch start as
        # soon as the framework preamble ends, releasing the HAM clock gate
        # before real data arrives.
        wsc = constp.tile([128, NPX], BF16, name="wsc")
        nc.gpsimd.memset(wsc.bitcast(U16), 0)
        wpsum = warmp.tile([128, NPX], F32, name="wpsum")
        for i in range(NWARM):
            nc.tensor.matmul(wpsum, wsc[:, :128], wsc,
                             start=(i == 0), stop=(i == NWARM - 1))

        # x arrives host-padded ([128, 66, 66] with zero borders baked in),
        # so band DMAs are fully contiguous on both sides (128 descriptors
        # instead of ~2200 strided 128B ones) and no border memsets exist.
        xpads = []
        for t in range(2):
            xp = xpadp.tile([128, XR * XP], BF16, name=f"xpad{t}",
                            tag=f"xpad{t}")
            xpads.append(xp.rearrange("p (r c) -> p r c", c=XP))

        # Input DMAs.  Completion-semaphore posting lags the data by ~1.5us
        # on a lightly-loaded HWDGE queue and 3-5us on a congested one, and
        # the first band + first wext block gate stage-1's first matmuls -
        # so the early transfers lead both queues (sync and scalar are the
        # only HWDGE queues).  x bands are DISJOINT (overlap would create
        # WAW deps that serialize the queue on completion semaphores) and
        # pair-aligned: stage-1 pair p reads xpad rows [16p, 16p+18), so
        # with splits at 18/34/50 pair p is gated by band p only.
        def band_dma(q, t, ra, rb):
            dst = xpads[t][:, ra:rb, :]
            src = x[t * 128:(t + 1) * 128, ra * XP:rb * XP]
            nc_q = getattr(nc, q)
            nc_q.dma_start(out=dst, in_=src.rearrange("p (r c) -> p r c", c=XP))

        BSPLIT = (0, 18, 34, 50, 66)

        def wext_dma(lo, hi, q="scalar"):
            getattr(nc, q).dma_start(out=wext[:, lo * 128:hi * 128],
                                     in_=wext_d[:, lo * 128:hi * 128])

        # Both first-matmul gates (band0 t0 + wext blocks 0-8) ride the
        # SAME queue back-to-back: their semaphores post ~0.65us apart, so
        # the start is one queue's bring-up time, not the max of two
        # independent ones.  t=1 transfers (read from matmul #19 of a
        # chunk) lead the scalar queue.
        band_dma("sync", 0, BSPLIT[0], BSPLIT[1])
        wext_dma(0, 9)
        band_dma("scalar", 1, BSPLIT[0], BSPLIT[1])
        wext_dma(9, 18)
        band_dma("sync", 0, BSPLIT[1], BSPLIT[2])
        band_dma("scalar", 1, BSPLIT[1], BSPLIT[2])
        nc.scalar.dma_start(out=bext, in_=bext_d)
        nc.scalar.dma_start(out=breg, in_=breg_d)
        band_dma("sync", 0, BSPLIT[2], BSPLIT[3])
        band_dma("scalar", 1, BSPLIT[2], BSPLIT[3])
        band_dma("sync", 0, BSPLIT[3], BSPLIT[4])
        band_dma("scalar", 1, BSPLIT[3], BSPLIT[4])
        nc.scalar.dma_start(out=wreg, in_=wreg_d)

        # ---- padded features (pad=2) + guard tail; borders on GpSimd ----
        fpad = fpadp.tile([128, FR * FP + FTAIL], BF16, name="fpad")
        fr = fpad[:, :FR * FP].rearrange("p (r c) -> p r c", c=FP)
        fi = fpad.bitcast(U16)
        fri = fi[:, :FR * FP].rearrange("p (r c) -> p r c", c=FP)
        nc.gpsimd.memset(fi[:, 0:2 * FP], 0)
        nc.gpsimd.memset(fi[:, (FR - 2) * FP:FR * FP + FTAIL], 0)
        nc.gpsimd.memset(fri[:, 2:FR - 2, 0:2], 0)
        nc.gpsimd.memset(fri[:, 2:FR - 2, FP - 2:FP], 0)

        # ---- stage 1: f = conv3x3(x) + b_ext ----
        # 18 accumulating matmuls (cin tile x 3x3 tap) per 512-pixel chunk;
        # each chunk's bias-copy overlaps the next chunk's matmuls.
        for c in range(NCHUNK):
            ps = ps1.tile([128, NPX], F32, name=f"ps1_{c}", tag="psum1")
            k = 0
            for t in range(2):
                for du in range(3):
                    for dv in range(3):
                        blk = t * 9 + du * 3 + dv
                        lhsT = wext[:, blk * 128:(blk + 1) * 128]
                        rhs = xpads[t][:,
                                       c * CROWS + du:
                                       c * CROWS + du + CROWS,
                                       dv:dv + W]
                        nc.tensor.matmul(ps, lhsT, rhs,
                                         start=(k == 0), stop=(k == 17))
                        k += 1
            dst = fr[:, c * CROWS + 2:c * CROWS + 2 + CROWS, 2:2 + W]
            nc.scalar.activation(dst,
                                 ps.rearrange("p (r c) -> p r c", c=W),
                                 AF.Identity, bias=bext, scale=1.0)

        # ---- stage 2: products (2-chunk groups) + regressor GEMM ----
        for g in range(NGRP):
            # product map for tap (a,b): rows [g*16+2-a, g*16+18) of the
            # (-2-origin) padded product grid, full FP-wide rows
            ptiles = []
            for kk, (a, b) in enumerate(SYM):
                nrows = GROWS + a if kk > 0 else GROWS
                base = (g * GROWS + 2 - (a if kk > 0 else 0)) * FP
                pt = prodp.tile([128, nrows * FP], BF16,
                                name=f"prod{kk}", tag=f"prod{kk}", bufs=2)
                in0 = fpad[:, base:base + nrows * FP]
                in1 = fpad[:, base + a * FP + b:
                           base + a * FP + b + nrows * FP]
                if kk == 0:
                    nc.scalar.activation(pt, in0, AF.Square)
                else:
                    nc.vector.tensor_mul(pt, in0, in1)
                ptiles.append(pt)

            # Regressor GEMM, chunk-outer: each chunk's bias-copy and out
            # DMA overlap the next chunk's GEMM.  bf16 weight loads hide
            # fully under the previous matmul, so per-tap weight reuse
            # buys nothing here.
            for i in (2 * g, 2 * g + 1):
                p8 = (i % 2) * CROWS
                psum2 = ps2.tile([COUT, NPX], F32, name=f"psum2_{i}",
                                 tag="psum2")
                mm = 0
                for kk, (a, b) in enumerate(SYM):
                    pr = ptiles[kk].rearrange("p (r c) -> p r c", c=FP)
                    taps = ([(a, b)] if (a, b) == (0, 0)
                            else [(a, b), (-a, -b)])
                    for (p, q) in taps:
                        if kk == 0:
                            rhs = pr[:, p8:p8 + CROWS, 2:2 + W]
                        elif (p, q) == (a, b):
                            rhs = pr[:, p8 + a:p8 + a + CROWS, 2:2 + W]
                        else:
                            rhs = pr[:, p8:p8 + CROWS, 2 - b:2 - b + W]
                        tidx = (p + 2) * 5 + (q + 2)
                        lhsT = wreg[:, tidx * 64:(tidx + 1) * 64]
                        nc.tensor.matmul(psum2, lhsT, rhs,
                                         start=(mm == 0), stop=(mm == 24))
                        mm += 1

                outt = outp.tile([COUT, NPX], F32, name="outsb",
                                 tag="outsb")
                # Final chunk: halve the copy+DMA so the first half's DMA
                # overlaps the second half's copy (shortens the serial tail
                # after the last matmul).
                halves = ((0, NPX // 2), (NPX // 2, NPX)) if i == 7                     else ((0, NPX),)
                for lo, hi in halves:
                    nc.scalar.activation(outt[:, lo:hi], psum2[:, lo:hi],
                                         AF.Identity, bias=breg, scale=1.0)
                    nc.sync.dma_start(
                        out=out[:, i * NPX + lo:i * NPX + hi],
                        in_=outt[:, lo:hi])


def build_nc():
    nc = bacc.Bacc("TRN2", target_bir_lowering=False, debug=False,
                   num_devices=NCORES)
    x = nc.dram_tensor("x", [CIN, XR * XP], BF16, kind="ExternalInput").ap()
    wext_d = nc.dram_tensor("wext", [128, 18 * 128], BF16,
                            kind="ExternalInput").ap()
    wreg_d = nc.dram_tensor("wreg", [128, 25 * 64], BF16,
                            kind="ExternalInput").ap()
    bext_d = nc.dram_tensor("bext", [128, 1], F32, kind="ExternalInput").ap()
    breg_d = nc.dram_tensor("breg", [64, 1], F32, kind="ExternalInput").ap()
    out = nc.dram_tensor("out", [COUT, HW], F32, kind="ExternalOutput").ap()
    with tile.TileContext(nc) as tc:
        with tc.tile_pool(name="weights", bufs=1) as wp:
            wext = wp.tile([128, 18 * 128], BF16, name="wext_sb")
            wreg = wp.tile([128, 25 * 64], BF16, name="wreg_sb")
            bext = wp.tile([128, 1], F32, name="bext_sb")
            breg = wp.tile([64, 1], F32, name="breg_sb")
            build_body(nc, tc, x, wext, wreg, bext, breg, out,
                       wext_d, wreg_d, bext_d, breg_d)
    nc.compile()
    return nc


def prep_in_maps(x, w_ext, b_ext, w_reg, b_reg):
    x = np.ascontiguousarray(np.asarray(x, dtype=np.float32))
    w_ext = np.asarray(w_ext, dtype=np.float32)
    w_reg = np.asarray(w_reg, dtype=np.float32)
    b_ext = np.asarray(b_ext, dtype=np.float32)
    b_reg = np.asarray(b_reg, dtype=np.float32)

    # lhsT layouts: wext [cin(128-part), (cintile,tap)*cc], wreg [cc, tap*cout]
    w1 = np.transpose(w_ext, (1, 2, 3, 0))          # [CIN, 3, 3, CC]
    wext_p = np.zeros((128, 18, 128), np.float32)
    for t in range(2):
        for du in range(3):
            for dv in range(3):
                wext_p[:, t * 9 + du * 3 + dv, :] = \
                    w1[t * 128:(t + 1) * 128, du, dv, :]
    wext_p = np.ascontiguousarray(wext_p.reshape(128, 18 * 128))
    w2 = np.transpose(w_reg, (1, 2, 3, 0))          # [CC, 5, 5, COUT]
    wreg_p = np.ascontiguousarray(w2.reshape(128, 25 * 64))
    bext_p = np.ascontiguousarray(b_ext.reshape(128, 1))
    breg_p = np.ascontiguousarray(b_reg.reshape(64, 1))

    bf = ml_dtypes.bfloat16
    wext_b = wext_p.astype(bf)
    wreg_b = wreg_p.astype(bf)

    # host-padded x: [CIN, 66, 66] with zero borders (pad=1) baked in
    xpad_h = np.zeros((B, CIN, XR, XP), bf)
    xpad_h[:, :, 1:1 + H, 1:1 + W] = x.reshape(B, CIN, H, W).astype(bf)
    xpad_h = np.ascontiguousarray(xpad_h.reshape(B, CIN, XR * XP))

    return [{
        "x": xpad_h[b],
        "wext": wext_b,
        "wreg": wreg_b,
        "bext": bext_p,
        "breg": breg_p,
    } for b in range(B)]


_NC_CACHE = None


def kernel(x, w_ext, b_ext, w_reg, b_reg):
    global _NC_CACHE
    if _NC_CACHE is None:
        _NC_CACHE = build_nc()
    nc = _NC_CACHE
    in_maps = prep_in_maps(x, w_ext, b_ext, w_reg, b_reg)
    res = run_bass_kernel_spmd(nc, in_maps, list(range(NCORES)))
    return np.stack([res.results[b]["out"].reshape(COUT, H, W)
                     for b in range(B)], axis=0)
